# revision 1
# baseline (speedup 1.0000x reference)
"""CropPool2D Trainium2 kernel.

out[b, c] = mean of img_feats[b, c, y1:y2, x1:x2] for bbox (x1, y1, x2, y2).

Strategy (data-parallel over batch, 8 NeuronCores, 8 samples each):
  - Host derives, per sample: a fixed-size crop window (Hw x Ww = max crop
    extents over the whole batch), its band start row offset, column offset,
    and a window mask with 1/area folded in (0 outside the crop).
  - Device, per (sample, channel-group-of-128):
      * one contiguous-band DMA: img[s, 128ch, ys:ys+Hw, :] (rows are
        contiguous in memory -> 56*4B-granule descriptors at line rate),
        with the row offset taken from a register (same SPMD program on
        every core; offsets are data).
      * one fused DVE tensor_tensor_reduce: (window * mask) summed over the
        free dim -> the [128,1] per-channel crop mean (mask carries 1/area).
  - One final DMA scatters the [128, samples*groups] result tile to the
    [samples, C] output.
"""

import numpy as np

B, C, H, W = 64, 512, 56, 56
N_CORES = 8
BL = B // N_CORES  # samples per core
P = 128
G = C // P  # channel groups per sample

_prog_cache: dict = {}

# Offset sentinel far past the img tensor end: the DMA bounds check sees the
# whole AP out of range and (with bounds_check="skip_entire_dma") skips the
# transfer while still incrementing its semaphore.
_SENTINEL = 1 << 24


def _chunking(Hw: int):
    """Split the Hw-row band into n_chunks of ch_rows rows for skip-DMA."""
    n_chunks = min(4, Hw)
    ch_rows = -(-Hw // n_chunks)
    n_chunks = -(-Hw // ch_rows)
    return n_chunks, ch_rows


def _build_program(Hw: int, Ww: int, unroll: int = 1):
    """Build + compile the SPMD Bass program for window size Hw x Ww.

    unroll > 1 repeats the whole body (idempotent) for benchmarking: the
    marginal wall time per extra repetition is the kernel's steady-state
    device time without dispatch overhead.
    """
    import concourse.bacc as bacc
    import concourse.mybir as mybir
    import concourse.tile as tile
    from concourse.bass import ds

    f32 = mybir.dt.float32
    i32 = mybir.dt.int32

    nc = bacc.Bacc("TRN2", target_bir_lowering=False, debug=False)

    n_chunks, ch_rows = _chunking(Hw)

    img = nc.dram_tensor("img", [BL, C, H, W], f32, kind="ExternalInput").ap()
    # meta: per (sample, chunk): window-start offset, then a 0/1 load flag.
    meta = nc.dram_tensor(
        "meta", [1, 2 * BL * n_chunks], i32, kind="ExternalInput"
    ).ap()
    maskd = nc.dram_tensor("mask", [BL, Hw * Ww], f32, kind="ExternalInput").ap()
    outd = nc.dram_tensor("out", [BL, C], f32, kind="ExternalOutput").ap()

    with tile.TileContext(nc) as tc:
        with (
            tc.tile_pool(name="const", bufs=1) as constp,
            tc.tile_pool(name="bandp", bufs=4) as bandp,
            tc.tile_pool(name="maskp", bufs=2) as maskp,
            tc.tile_pool(name="prodp", bufs=2) as prodp,
            tc.tile_pool(name="outp", bufs=1) as outp,
        ):
            meta_sb = constp.tile([1, 2 * BL * n_chunks], i32)
            nc.sync.dma_start(meta_sb, meta)

            out_sb = outp.tile([P, BL * G], f32)

            # Pre-zero the band slots: chunk DMAs skipped at runtime leave
            # slot contents stale; zeroing once guarantees the masked-out
            # region is finite (0 * 0 = 0) even on first use.
            for _ in range(4):
                t = bandp.tile([P, Hw, W], f32, tag="band")
                nc.any.memset(t, 0.0)

            n_off = BL * n_chunks
            img_flat = img.rearrange("b c h w -> b c (h w)")
            # Chunk DMAs alternate between the two HWDGE rings (SP, ACT);
            # each ring's offset/flag registers live on its own engine.
            ring_eng = [
                (nc.sync, (mybir.EngineType.SP,)),
                (nc.scalar, (mybir.EngineType.Activation,)),
            ]

            for _rep in range(unroll):
                offs = []
                flags = []
                for s in range(BL):
                    for k in range(n_chunks):
                        i = s * n_chunks + k
                        eng = ring_eng[k % 2][1]
                        offs.append(
                            nc.values_load(
                                meta_sb[0:1, i : i + 1],
                                engines=eng,
                                min_val=0,
                                max_val=(H - Hw) * W + (W - Ww) + k * ch_rows * W,
                                skip_runtime_bounds_check=True,
                            )
                        )
                        flags.append(
                            nc.values_load(
                                meta_sb[0:1, n_off + i : n_off + i + 1],
                                engines=eng,
                                min_val=0,
                                max_val=1,
                                skip_runtime_bounds_check=True,
                            )
                        )
                for s in range(BL):
                    mask_sb = maskp.tile([P, Hw * Ww], f32)
                    nc.sync.dma_start(
                        mask_sb, maskd[s : s + 1, :].to_broadcast((P, Hw * Ww))
                    )
                    mask_v = mask_sb[:].rearrange("p (r x) -> p r x", x=Ww)
                    for g in range(G):
                        band = bandp.tile([P, Hw, W], f32, tag="band")
                        chan = img_flat[s, g * P : (g + 1) * P, :]
                        for k in range(n_chunks):
                            r0 = k * ch_rows
                            r1 = min(r0 + ch_rows, Hw)
                            # Last chunk stops at the window's last element so
                            # the span never crosses the channel end.
                            span = (
                                (r1 - r0) * W if r1 < Hw else (r1 - r0 - 1) * W + Ww
                            )
                            dst = band[:, r0:r1, :].rearrange("p r x -> p (r x)")[
                                :, 0:span
                            ]
                            i = s * n_chunks + k
                            ring_eng[k % 2][0].dma_start(
                                dst,
                                chan[:, ds(offs[i], span)],
                                cond=flags[i],
                            )

                        prod = prodp.tile([P, Hw * Ww], f32)
                        prod_v = prod[:].rearrange("p (r x) -> p r x", x=Ww)

                        col = s * G + g
                        # out = (window * 1.0) * mask; accum_out = sum(out).
                        # Window is the static strided view: rows stride W, cols 1.
                        nc.vector.scalar_tensor_tensor(
                            out=prod_v,
                            in0=band[:, :, 0:Ww],
                            scalar=1.0,
                            in1=mask_v,
                            op0=mybir.AluOpType.mult,
                            op1=mybir.AluOpType.mult,
                            accum_out=out_sb[:, col : col + 1],
                        )

            nc.sync.dma_start(
                outd.rearrange("s (g p) -> p s g", p=P),
                out_sb[:].rearrange("p (s g) -> p s g", g=G),
            )

    nc.compile()
    return nc


def _host_prep(img_feats: np.ndarray, bboxes: np.ndarray):
    bb = np.asarray(bboxes).astype(np.int64)
    x1, y1, x2, y2 = bb[:, 0], bb[:, 1], bb[:, 2], bb[:, 3]
    ch = y2 - y1
    cw = x2 - x1
    assert (ch > 0).all() and (cw > 0).all(), "invalid bboxes"
    Hw = int(ch.max())
    Ww = int(cw.max())
    ys = np.minimum(y1, H - Hw)
    xs = np.minimum(x1, W - Ww)
    dy = y1 - ys
    dx = x1 - xs
    inv_area = (1.0 / (ch * cw)).astype(np.float64)

    r = np.arange(Hw)[None, :, None]
    c = np.arange(Ww)[None, None, :]
    valid = (
        (r >= dy[:, None, None])
        & (r < (dy + ch)[:, None, None])
        & (c >= dx[:, None, None])
        & (c < (dx + cw)[:, None, None])
    )
    masks = (valid * inv_area[:, None, None]).astype(np.float32).reshape(B, Hw * Ww)

    # Per-chunk window-start offsets plus 0/1 flags; chunks with no valid
    # rows get flag 0 so the device DMA is predicated off (cond=).
    n_chunks, ch_rows = _chunking(Hw)
    base = ys * W + xs  # [B]
    offsets = np.empty((B, n_chunks), np.int64)
    flags = np.empty((B, n_chunks), np.int64)
    for k in range(n_chunks):
        r0, r1 = k * ch_rows, min((k + 1) * ch_rows, Hw)
        needed = (r0 < dy + ch) & (r1 > dy)  # chunk overlaps valid rows
        offsets[:, k] = base + r0 * W
        flags[:, k] = needed
    # Per-core rows [N_CORES, 2*BL*n_chunks]: that core's sample offsets
    # flattened, then its flags.
    meta = np.concatenate(
        [
            offsets.reshape(N_CORES, BL * n_chunks),
            flags.reshape(N_CORES, BL * n_chunks),
        ],
        axis=1,
    ).astype(np.int32)
    return Hw, Ww, masks, meta


def _run(img_feats: np.ndarray, bboxes: np.ndarray, **spmd_kwargs):
    from concourse.bass_utils import run_bass_kernel_spmd

    img = np.ascontiguousarray(np.asarray(img_feats), dtype=np.float32)
    assert img.shape == (B, C, H, W), img.shape
    Hw, Ww, masks, meta = _host_prep(img, bboxes)

    key = (Hw, Ww)
    if key not in _prog_cache:
        _prog_cache[key] = _build_program(Hw, Ww)
    nc = _prog_cache[key]

    in_maps = []
    for i in range(N_CORES):
        sl = slice(i * BL, (i + 1) * BL)
        in_maps.append(
            {
                "img": img[sl],
                "meta": meta[i : i + 1],
                "mask": masks[sl],
            }
        )

    res = run_bass_kernel_spmd(
        nc, in_maps, core_ids=list(range(N_CORES)), **spmd_kwargs
    )
    out = np.concatenate([res.results[i]["out"] for i in range(N_CORES)], axis=0)
    return out.astype(np.float32), res


def kernel(img_feats: np.ndarray, bboxes: np.ndarray) -> np.ndarray:
    out, _ = _run(img_feats, bboxes)
    return out

